# revision 1
# baseline (speedup 1.0000x reference)
"""Weighted 2D cross-entropy (BCE-over-classes) loss on 8 Trainium2 cores.

Math (matches the reference):
  t in [0,19); pos = t>0, neg = t==0 (all pixels are pos or neg; mask == 1)
  S(i) = sum_c bce(i,c) = -[ B(i) + A(i) ]
     A(i) = sum_c log(1-p_c(i))
     B(i) = log(p_t(i)) - log(1-p_t(i))
  loss = ( (NEG/TOT)*S_pos_sum + (POS/TOT)*S_neg_sum ) / (TOT*C)

Per-core (core k <- batch element k, pure data parallel):
  - one full-grid ACT pass: L_c = Ln(1-p_c) in bf16, accum_out -> U_all
  - one full-grid DVE pass: masked_c = (T==c)*L_c (fused scalar_tensor_tensor)
  - PE identity-matmuls accumulate A = sum_c L_c and L_sel = sum_c masked_c
    into PSUM (f32) -- the gather of log(1-p) at the target class.
  - per-pixel tail: B = log(1-exp(L_sel)) - L_sel on ACT; masked sums via
    accum_out; final partition reduce via ones-matmul -> 8 scalars per core.
Host combines the 8x8 scalars in float64 (the "all-reduce").
"""

from contextlib import ExitStack

import numpy as np

import concourse.bass as bass
import concourse.mybir as mybir
import concourse.tile as tile
from concourse import bacc
from concourse.bass_utils import run_bass_kernel_spmd

# problem shape (hardcoded per harness contract)
N, C, H, W = 8, 19, 512, 1024
PIX = H * W          # 524288 pixels per core
P = 128              # partitions
FCOLS = PIX // P     # 4096 free columns when pixels laid out [128, 4096]
FT = 1024            # pixel-tile free width
NTILES = FCOLS // FT # 4 pixel tiles per core
N_CORES = 8

DT = mybir.dt

# stats buffer column layout (all f32; per-tile partial sums, 4 cols per group)
#   sum A (=U_all), sum logp, sum pos*A, sum pos*logp, sum pos*L_sel,
#   sum L_sel, pos count
COL_UALL = 0
COL_LOGP = 4
COL_POSA = 8
COL_POSLOGP = 12
COL_POSLSEL = 16
COL_LSEL = 20
COL_CNT = 24
NSTAT = 7  # number of final scalars (one per group above)
STAT_COLS = 32


def build_kernel() -> bass.Bass:
    # Bacc (not raw Bass): its compile() pipeline runs
    # generate_event_semaphores, which splits multi-sem waits to satisfy the
    # 1-wait-per-instruction TRN2 sync structs -- raw Bass modules with
    # Tile-emitted multi-waits fail walrus codegen.
    nc = bacc.Bacc("TRN2")

    predict = nc.declare_dram_parameter("predict", [C, PIX], DT.float32, isOutput=False)
    target = nc.declare_dram_parameter("target", [P, FCOLS], DT.int32, isOutput=False)
    idn = nc.declare_dram_parameter("idn", [P, P], DT.bfloat16, isOutput=False)
    out = nc.declare_dram_parameter("out", [1, NSTAT], DT.float32, isOutput=True)

    pred_r = predict.rearrange("c (p f) -> c p f", p=P)  # [19, 128, 4096]

    with tile.TileContext(nc) as tc, ExitStack() as ctx:
        const = ctx.enter_context(tc.tile_pool(name="const", bufs=1))
        p_pool = ctx.enter_context(tc.tile_pool(name="p", bufs=8))
        lm_pool = ctx.enter_context(tc.tile_pool(name="lm", bufs=21))
        pix_pool = ctx.enter_context(tc.tile_pool(name="pix", bufs=2))
        scr_pool = ctx.enter_context(tc.tile_pool(name="scr", bufs=2))
        eq_pool = ctx.enter_context(tc.tile_pool(name="eq", bufs=4))
        psum_pool = ctx.enter_context(tc.tile_pool(name="ps", bufs=2, space="PSUM"))

        idn_sb = const.tile([P, P], DT.bfloat16, tag="idn")
        nc.sync.dma_start(out=idn_sb[:], in_=idn[:])

        t_i32 = const.tile([P, FCOLS], DT.int32, tag="ti")
        nc.sync.dma_start(out=t_i32[:], in_=target[:])
        t_bf = const.tile([P, FCOLS], DT.bfloat16, tag="tb")
        nc.vector.tensor_copy(out=t_bf[:], in_=t_i32[:])

        stats = const.tile([P, STAT_COLS], DT.float32, tag="stats")
        nc.vector.memset(stats[:], 0.0)

        # pos counts up-front (also settles the DVE self-dep on t_bf so later
        # scalar_tensor_tensor ops carry at most one sem wait -- the STT
        # hardware sync struct only holds a single wait condition)
        cnt_scr = const.tile([P, FT], DT.bfloat16, tag="cntscr")
        for t in range(NTILES):
            nc.vector.tensor_scalar(
                out=cnt_scr[:],
                in0=t_bf[:, t * FT : (t + 1) * FT],
                scalar1=0.5,
                scalar2=None,
                op0=mybir.AluOpType.is_gt,
                op1=mybir.AluOpType.add,
                accum_out=stats[:, COL_CNT + t : COL_CNT + t + 1],
            )

        for t in range(NTILES):
            fsl = slice(t * FT, (t + 1) * FT)
            t_sl = t_bf[:, fsl]

            # PSUM accumulator: [:, :FT] = A, [:, FT:] = L_sel   (4 banks)
            acc_ps = psum_pool.tile([P, 2 * FT], DT.float32, tag="acc")

            for c in range(C):
                p_t = p_pool.tile([P, FT], DT.float32, tag="p")
                # p bufs=8 aligns slot reuse with the global DMA->DMAHW-proc
                # round-robin (8 procs), so the WAW on the old writer is
                # same-proc FIFO order and Tile emits no cross-queue wait
                nc.sync.dma_start(out=p_t[:], in_=pred_r[c, :, fsl])

                # lm[:, :FT] = L_c = Ln(1-p) bf16 ; lm[:, FT:] = (T==c)*L_c
                lm = lm_pool.tile([P, 2 * FT], DT.bfloat16, tag="lm")
                nc.scalar.activation(
                    out=lm[:, :FT],
                    in_=p_t[:],
                    func=mybir.ActivationFunctionType.Ln,
                    bias=1.0,
                    scale=-1.0,
                )
                # eq at DVE 4x (16-bit tensor_scalar) + mult at 2x beats the
                # fused scalar_tensor_tensor, which only has a 1x uop
                eq = eq_pool.tile([P, FT], DT.bfloat16, tag="eq")
                nc.vector.tensor_scalar(
                    out=eq[:],
                    in0=t_sl,
                    scalar1=float(c),
                    scalar2=None,
                    op0=mybir.AluOpType.is_equal,
                )
                nc.vector.tensor_mul(out=lm[:, FT:], in0=eq[:], in1=lm[:, :FT])

                for s in range(4):
                    ssl = slice(s * 512, (s + 1) * 512)
                    nc.tensor.matmul(
                        acc_ps[:, ssl],
                        lhsT=idn_sb[:],
                        rhs=lm[:, ssl],
                        start=(c == 0),
                        stop=(c == C - 1),
                    )

            a_ps = acc_ps[:, :FT]
            lsel_ps = acc_ps[:, FT:]

            # expL = exp(L_sel) = 1-p_t ;  logp = Ln(1 - expL) = log(p_t)
            expl = pix_pool.tile([P, FT], DT.float32, tag="expl")
            nc.scalar.activation(
                out=expl[:], in_=lsel_ps, func=mybir.ActivationFunctionType.Exp
            )
            logp = pix_pool.tile([P, FT], DT.float32, tag="logp")
            nc.scalar.activation(
                out=logp[:],
                in_=expl[:],
                func=mybir.ActivationFunctionType.Ln,
                bias=1.0,
                scale=-1.0,
            )
            # sum A  (= U_all contribution)
            nc.vector.tensor_reduce(
                out=stats[:, COL_UALL + t : COL_UALL + t + 1],
                in_=a_ps,
                axis=mybir.AxisListType.X,
                op=mybir.AluOpType.add,
            )
            # sum logp
            nc.vector.tensor_reduce(
                out=stats[:, COL_LOGP + t : COL_LOGP + t + 1],
                in_=logp[:],
                axis=mybir.AxisListType.X,
                op=mybir.AluOpType.add,
            )

            scr = scr_pool.tile([P, FT], DT.float32, tag="scr")
            # sum pos*A
            nc.vector.scalar_tensor_tensor(
                out=scr[:],
                in0=t_sl,
                scalar=0.5,
                in1=a_ps,
                op0=mybir.AluOpType.is_gt,
                op1=mybir.AluOpType.mult,
                accum_out=stats[:, COL_POSA + t : COL_POSA + t + 1],
            )
            # sum pos*logp
            nc.vector.scalar_tensor_tensor(
                out=scr[:],
                in0=t_sl,
                scalar=0.5,
                in1=logp[:],
                op0=mybir.AluOpType.is_gt,
                op1=mybir.AluOpType.mult,
                accum_out=stats[:, COL_POSLOGP + t : COL_POSLOGP + t + 1],
            )
            # sum pos*L_sel
            nc.vector.scalar_tensor_tensor(
                out=scr[:],
                in0=t_sl,
                scalar=0.5,
                in1=lsel_ps,
                op0=mybir.AluOpType.is_gt,
                op1=mybir.AluOpType.mult,
                accum_out=stats[:, COL_POSLSEL + t : COL_POSLSEL + t + 1],
            )
            # sum L_sel
            nc.vector.tensor_reduce(
                out=stats[:, COL_LSEL + t : COL_LSEL + t + 1],
                in_=lsel_ps,
                axis=mybir.AxisListType.X,
                op=mybir.AluOpType.add,
            )

        # fold each stat group into one column, then partition-reduce via matmul
        finals = const.tile([P, NSTAT], DT.float32, tag="finals")
        groups = [
            (COL_UALL, NTILES),
            (COL_LOGP, NTILES),
            (COL_POSA, NTILES),
            (COL_POSLOGP, NTILES),
            (COL_POSLSEL, NTILES),
            (COL_LSEL, NTILES),
            (COL_CNT, NTILES),
        ]
        for g, (start, width) in enumerate(groups):
            nc.vector.tensor_reduce(
                out=finals[:, g : g + 1],
                in_=stats[:, start : start + width],
                axis=mybir.AxisListType.X,
                op=mybir.AluOpType.add,
            )

        out_sb = const.tile([1, NSTAT], DT.float32, tag="outsb")
        nc.gpsimd.tensor_reduce(
            out=out_sb[:],
            in_=finals[:],
            axis=mybir.AxisListType.C,
            op=mybir.AluOpType.add,
        )
        nc.gpsimd.dma_start(out=out[:], in_=out_sb[:])

    if not nc.is_finalized():
        nc.finalize()

    return nc
    for f in nc.m.functions:
        for bb in f.blocks:
            il = bb.instructions
            i = 0
            n_split = 0
            while i < len(il):
                ins = il[i]
                i += 1
                if ins.opcode == "Drain" or ins.sync_info is None:
                    continue
                w = ins.sync_info.on_wait
                if not w or len(w) < 2:
                    continue
                if ins.opcode == "DMACopy" and len(w) == 2:
                    act = [x for x in w if x.ant_name.startswith("Activation")]
                    hw = [x for x in w if x.ant_name.startswith("DMAHW")]
                    if len(act) == 1 and len(hw) == 1:
                        ins.sync_info = mybir.SyncInfo(
                            on_wait=act, on_update=ins.sync_info.on_update
                        )
                        continue
                for j, extra in enumerate(list(w)[:-1]):
                    drain = mybir.InstDrain(
                        name=f"{ins.name}-waitsplit{j}",
                        engine=ins.engine,
                        sync_info=mybir.SyncInfo(on_wait=[extra], on_update=[]),
                    )
                    il.insert(i - 1, drain)
                    i += 1
                ins.sync_info = mybir.SyncInfo(
                    on_wait=[w[-1]], on_update=ins.sync_info.on_update
                )
                n_split += 1

    return nc


_NC_CACHE = None


def kernel(predict: np.ndarray, target: np.ndarray) -> np.ndarray:
    global _NC_CACHE
    if _NC_CACHE is None:
        _NC_CACHE = build_kernel()
    nc = _NC_CACHE

    import ml_dtypes

    predict = np.ascontiguousarray(predict, dtype=np.float32)
    target = np.ascontiguousarray(target, dtype=np.int32)
    idn = np.eye(P, dtype=np.float32).astype(ml_dtypes.bfloat16)

    in_maps = []
    for k in range(N_CORES):
        in_maps.append(
            {
                "predict": predict[k].reshape(C, PIX),
                "target": target[k].reshape(P, FCOLS),
                "idn": idn,
            }
        )

    res = run_bass_kernel_spmd(nc, in_maps, list(range(N_CORES)))

    tot = np.float64(0.0)
    s_all = np.float64(0.0)
    s_pos = np.float64(0.0)
    pos = np.float64(0.0)
    for k in range(N_CORES):
        st = res.results[k]["out"].reshape(-1).astype(np.float64)
        u_all, logp_s, pos_a, pos_logp, pos_lsel, lsel_s, cnt = st[:NSTAT]
        v_all = logp_s - lsel_s
        v_pos = pos_logp - pos_lsel
        s_all += -(v_all + u_all)
        s_pos += -(v_pos + pos_a)
        pos += cnt
        tot += PIX
    neg = tot - pos
    s_neg = s_all - s_pos
    loss = ((neg / tot) * s_pos + (pos / tot) * s_neg) / (tot * C)
    return np.float32(loss)

